# revision 8
# baseline (speedup 1.0000x reference)
"""TRN2 Bass kernel for nn_MindieCifTimestamp (CIF no-hidden scan).

reference:  acc' = acc + a_t;  fire_t = acc';  acc' -= (acc' >= 1.0)
i.e. mod-1 cumulative summation per batch row.

Data-parallel over 8 NeuronCores (512 rows each). Per 128-row group,
streamed in W-column blocks (chunk C):
  tails = segmented-reduce(a)                   [chunk sums]
  chain: frac-accumulate tails -> c_k in [0,1)  [hierarchical, small ops]
  X: per-chunk hardware scan with initial=c_k   [unwrapped accumulator]
  G = floor(X) via ScalarE int32 cast (bias -0.5); zero G at chunk tails
  fire = X - G_shifted  (TensorE identity-matmuls into PSUM, ScalarE
         copies back; exact subtraction)
  integrate_new = frac(X_tail) at the last block

Numerics: local rounding/threshold flips only (no sequential error
propagation): fires rel-l2 ~4.4e-3 vs the f32 reference, integrate_new
~7e-5 (measured on the key(0) dataset).
"""
import os
_p = os.environ.get("JAX_PLATFORMS", "")
if "axon" not in _p.split(","):
    os.environ["JAX_PLATFORMS"] = "axon,cpu" if not _p else _p + ",axon"

import numpy as np

import concourse.bass as bass
import concourse.mybir as mybir
import concourse.tile as tile
from concourse.bass_utils import run_bass_kernel_spmd
from concourse.masks import make_identity

F32 = mybir.dt.float32
I32 = mybir.dt.int32
AL = mybir.AluOpType

B, T = 4096, 8192
NCORES = 8
ROWS = B // NCORES            # 512
W = 4096                      # column block
C = 64                        # chain chunk (chunk-sum / base granularity)
CS = 256                      # scan chunk (X unwraps over CS columns)
FIRE_PE = False               # fire subtract on TensorE (else DVE STT)


# ---------------------------------------------------------------- tile patch
# This walrus build encodes at most ONE sync-wait per instruction; Tile's
# scheduler freely attaches several (cross-engine deps, kernel-tail drain).
# Redistribute: single-wait nops ahead of multi-wait instructions, and a
# bare tail drain.
def _patched_drain_and_barrier(self, tick_clock, wait_clock):
    from bass_rust import ScopedClock
    nc = self.nc
    probe = nc.sync.nop()
    wait_clock.add_sem_waits(probe.ins, ScopedClock({None: tick_clock.global_clock}))
    si = probe.ins.sync_info
    waits = list(si.on_wait or []) if si is not None else []
    if len(waits) > 1:
        si.on_wait = waits[:1]
        for w in waits[1:]:
            extra = nc.sync.nop()
            esi = extra.ins.sync_info
            if esi is None:
                extra.ins.sync_info = type(si)(on_wait=[w], on_update=[])
            else:
                esi.on_wait = [w]
    nc.sync.drain()
    nc.all_engine_barrier()
    assert self.sems is not None
    popped = nc._tile_sem_poison_stack.pop()
    assert popped is self._sem_poison
    nc.clear_and_free_semaphores(list(self.sems.allocated().values()))
    nc.all_engine_barrier()


tile.TileContext._drain_and_barrier = _patched_drain_and_barrier


def _split_multi_waits(nc):
    seen = set()
    for name, bassbb in nc.bb_map.items():
        bb = bassbb.bb if hasattr(bassbb, "bb") else bassbb
        if id(bb) in seen:
            continue
        seen.add(id(bb))
        insts = list(bb.instructions)
        out = []
        changed = False
        for inst in insts:
            si = inst.sync_info
            waits = list(si.on_wait or []) if si is not None else []
            if len(waits) > 1:
                changed = True
                for w in waits[:-1]:
                    nop = mybir.InstNoOp(name=nc.get_next_instruction_name())
                    nop.engine = inst.engine
                    nop.sync_info = type(si)(on_wait=[w], on_update=[])
                    nc.register_instruction(nop)
                    out.append(nop)
                si.on_wait = waits[-1:]
            out.append(inst)
        if changed:
            bb.instructions = out


# ---------------------------------------------------------------- builder
def _block(nc, big, small, consts, a, fire_out_ap, base_ap, last, u):
    K = W // C
    neghalf = consts["neghalf"]

    tails = small.tile([128, K], F32, name=f"tl{u}", tag="tails")
    a3 = a[:, :].rearrange("p (k c) -> p k c", c=C)
    nc.vector.tensor_reduce(tails[:, :], a3, axis=mybir.AxisListType.X,
                            op=AL.add)

    # chain: ft = frac(tails); p2 = cumsum(ft); c = frac(base + shift(p2))
    ti = small.tile([128, K], I32, name=f"ti{u}", tag="ti")
    nc.vector.tensor_scalar(out=ti[:, :], in0=tails[:, :], scalar1=0.5,
                            scalar2=None, op0=AL.subtract)
    tf = small.tile([128, K], F32, name=f"tf{u}", tag="tf")
    nc.vector.tensor_copy(tf[:, :], ti[:, :])
    ft = small.tile([128, K], F32, name=f"ftl{u}", tag="ft")
    nc.vector.tensor_sub(ft[:, :], tails[:, :], tf[:, :])
    p2 = small.tile([128, K], F32, name=f"p2{u}", tag="p2")
    nc.vector.tensor_tensor_scan(out=p2[:, :], data0=ft[:, :], data1=ft[:, :],
                                 initial=0.0, op0=AL.add, op1=AL.bypass)
    cpre = small.tile([128, K], F32, name=f"cp{u}", tag="cpre")
    nc.vector.tensor_copy(cpre[:, 0:1], base_ap)
    nc.vector.tensor_scalar(out=cpre[:, 1:K], in0=p2[:, 0:K - 1],
                            scalar1=base_ap, scalar2=None, op0=AL.add)
    ci = small.tile([128, K], I32, name=f"ci{u}", tag="ci")
    nc.vector.tensor_scalar(out=ci[:, :], in0=cpre[:, :], scalar1=0.5,
                            scalar2=None, op0=AL.subtract)
    cf = small.tile([128, K], F32, name=f"cf{u}", tag="cf")
    nc.vector.tensor_copy(cf[:, :], ci[:, :])
    c = small.tile([128, K], F32, name=f"c{u}", tag="c")
    nc.vector.tensor_sub(c[:, :], cpre[:, :], cf[:, :])

    nb_ = small.tile([128, 1], F32, name=f"nb{u}", tag="nb")
    nc.vector.tensor_add(nb_[:, :], p2[:, K - 1:K], base_ap)
    nbi = small.tile([128, 1], I32, name=f"ni{u}", tag="nbi")
    nc.vector.tensor_scalar(out=nbi[:, :], in0=nb_[:, :], scalar1=0.5,
                            scalar2=None, op0=AL.subtract)
    nbf = small.tile([128, 1], F32, name=f"nf{u}", tag="nbf")
    nc.vector.tensor_copy(nbf[:, :], nbi[:, :])
    newbase = small.tile([128, 1], F32, name=f"bb{u}", tag="base")
    nc.vector.tensor_sub(newbase[:, :], nb_[:, :], nbf[:, :])

    # X = unwrapped accumulator via per-scan-chunk scans, seeded with the
    # chain base at each CS boundary (CS is a multiple of the chain chunk C;
    # only every (CS//C)-th base is consumed)
    x = big.tile([128, W], F32, name=f"x{u}", tag="x")
    step = CS // C
    for k in range(W // CS):
        s_ = slice(k * CS, (k + 1) * CS)
        nc.vector.tensor_tensor_scan(
            out=x[:, s_], data0=a[:, s_], data1=a[:, s_],
            initial=c[:, k * step:k * step + 1], op0=AL.add, op1=AL.bypass)

    gi = big.tile([128, W], I32, name=f"gi{u}", tag="gi")
    nc.scalar.activation(gi[:, :], x[:, :],
                         mybir.ActivationFunctionType.Identity,
                         bias=neghalf[:, 0:1], scale=1.0)

    rtail_ap = None
    if last:
        gt = small.tile([128, 1], F32, name=f"gt{u}", tag="gt")
        nc.vector.tensor_copy(gt[:, :], gi[:, W - 1:W])
        rt = small.tile([128, 1], F32, name=f"rt{u}", tag="rt")
        nc.vector.tensor_sub(rt[:, :], x[:, W - 1:W], gt[:, :])
        rtail_ap = rt[:, :]

    # zero scan-chunk-tail floors: scan-chunk-start fires then subtract 0
    nc.vector.memset(gi[:, CS - 1::CS], 0)

    if FIRE_PE:
        gf = big.tile([128, W], F32, name=f"gf{u}", tag="gf")
        nc.scalar.activation(gf[:, :], gi[:, :],
                             mybir.ActivationFunctionType.Copy)
        ident, nident = consts["ident"], consts["nident"]
        psum_pool = consts["psum_pool"]
        NCH = (W - 1 + 511) // 512
        for j in range(NCH):
            lo = 1 + j * 512
            hi = min(W, lo + 512)
            ps = psum_pool.tile([128, hi - lo], F32, name=f"ps{u}_{j}",
                                tag="ps")
            nc.tensor.matmul(ps[:, :], ident[:, :], x[:, lo:hi],
                             start=True, stop=False)
            nc.tensor.matmul(ps[:, :], nident[:, :], gf[:, lo - 1:hi - 1],
                             start=False, stop=True)
            nc.scalar.activation(x[:, lo:hi], ps[:, :],
                                 mybir.ActivationFunctionType.Copy)
    else:
        nc.vector.scalar_tensor_tensor(
            out=x[:, 1:W], in0=x[:, 1:W], scalar=0.0, in1=gi[:, 0:W - 1],
            op0=AL.add, op1=AL.subtract)

    nc.sync.dma_start(out=fire_out_ap, in_=x[:, :])
    return newbase[:, :], rtail_ap


def _build():
    G_ROWS = ROWS // 128
    NB = T // W
    nc = bass.Bass()
    alphas = nc.dram_tensor("us_alphas", [ROWS, T], F32, kind="ExternalInput")
    integ = nc.dram_tensor("integrate", [ROWS], F32, kind="ExternalInput")
    fires = nc.dram_tensor("fires", [ROWS, T], F32, kind="ExternalOutput")
    integ_new = nc.dram_tensor("integ_new", [ROWS], F32, kind="ExternalOutput")
    integ2d = integ[:].rearrange("(g p o) -> g p o", g=G_ROWS, o=1)
    inew2d = integ_new[:].rearrange("(g p o) -> g p o", g=G_ROWS, o=1)

    with tile.TileContext(nc) as tc:
        with (
            tc.tile_pool(name="big", bufs=3) as big,
            tc.tile_pool(name="small", bufs=2) as small,
            tc.tile_pool(name="consts", bufs=1) as consts_pool,
            tc.tile_pool(name="pspool", bufs=4, space="PSUM") as psum_pool,
        ):
            neghalf = consts_pool.tile([128, 1], F32, name="neghalf")
            nc.vector.memset(neghalf[:, :], -0.5)
            consts = {"neghalf": neghalf, "psum_pool": psum_pool}
            if FIRE_PE:
                ident = consts_pool.tile([128, 128], F32, name="ident")
                make_identity(nc, ident[:, :])
                nident = consts_pool.tile([128, 128], F32, name="nident")
                nc.vector.tensor_scalar_mul(nident[:, :], ident[:, :], -1.0)
                consts.update(ident=ident, nident=nident)

            for g in range(G_ROWS):
                base = small.tile([128, 1], F32, name=f"b{g}", tag="base")
                nc.sync.dma_start(out=base[:, :], in_=integ2d[g])
                for b in range(NB):
                    a = big.tile([128, W], F32, name=f"a{g}_{b}", tag="a")
                    nc.sync.dma_start(
                        out=a[:, :],
                        in_=alphas[g * 128:(g + 1) * 128, b * W:(b + 1) * W])
                    out_ap = fires[g * 128:(g + 1) * 128, b * W:(b + 1) * W]
                    base, rtail = _block(
                        nc, big, small, consts, a, out_ap, base,
                        last=(b == NB - 1), u=f"{g}_{b}")
                nc.sync.dma_start(out=inew2d[g], in_=rtail)

    _split_multi_waits(nc)
    return nc


_CACHED = {}


def _get_nc():
    key = (ROWS, T, W, C, CS, FIRE_PE)
    if key not in _CACHED:
        _CACHED[key] = _build()
    return _CACHED[key]


def kernel(us_alphas, integrate, _want_results_obj=False):
    us_alphas = np.ascontiguousarray(np.asarray(us_alphas, dtype=np.float32))
    integrate = np.ascontiguousarray(np.asarray(integrate, dtype=np.float32))
    assert us_alphas.shape == (B, T) and integrate.shape == (B,)

    nc = _get_nc()
    in_maps = [
        {
            "us_alphas": us_alphas[i * ROWS:(i + 1) * ROWS],
            "integrate": integrate[i * ROWS:(i + 1) * ROWS],
        }
        for i in range(NCORES)
    ]
    res = run_bass_kernel_spmd(nc, in_maps, list(range(NCORES)))

    fires = np.concatenate([res.results[i]["fires"] for i in range(NCORES)],
                           axis=0)
    integ_new = np.concatenate(
        [res.results[i]["integ_new"] for i in range(NCORES)], axis=0)
    if _want_results_obj:
        return (fires, integ_new), res
    return fires, integ_new
